# revision 6
# baseline (speedup 1.0000x reference)
"""Trainium2 Bass kernel for nn_ConvolutionLayer (5x5 VALID conv) — v5.

Full inputs:  x (16,32,224,224) f32, weight (64,32,5,5) f32, bias (64,) f32
Full output:  (16,64,220,220) f32
Sharding: data-parallel over batch — 2 images per core on 8 cores.

v5 = v4 with 440-column packing: images packed as n*220+w (valid
columns only) in BOTH pre-shifted banks, so matmuls stream 440 cols
instead of 448 and psum/stage carry no dead columns. kw=4 comes from
its own pre-shifted bank xg4 (rows grouped in 4-row blocks, shift 4
applied on host) instead of a padded grid read. 8 matmuls x 440 cols
per output-row pair; slab-batched DMAs; one 440-wide ACT per pair;
GH=20; xu on the sync ring, xg4/weights/stores on the scalar ring.
"""
import sys

sys.path.insert(0, "/opt/trn_rl_repo")

import numpy as np
import concourse.bacc as bacc
import concourse.mybir as mybir
from concourse.tile import TileContext
from concourse.bass_utils import run_bass_kernel_spmd

F32 = mybir.dt.float32
BF16 = mybir.dt.bfloat16
BF16NP = np.dtype("bfloat16")

N_CORES = 8
B, C, H, W = 16, 32, 224, 224
F, K = 64, 5
HO, WO = H - K + 1, W - K + 1  # 220, 220
NB = 2
NBLK = H // 4               # 56 row-quad blocks (kw=4 bank)
NW = NB * WO                # 440 packed data columns (valid w only)
GH = 10                     # output rows per staging/DMA group (5 pairs)
RU = 8                      # xu rows per slab DMA
NSU = H // RU               # 28 xu slabs
RB = 7                      # xg blocks per slab DMA
NSB = NBLK // RB            # 8 xg slabs

_cache = {}


def _build(reps=1, ubufs=5, gbufs=4, psbufs=8, stbufs=2, loop_n=0):
    nc = bacc.Bacc(trn_type="TRN2")

    xg4 = nc.dram_tensor("xg4", [128, NBLK, NW], BF16, kind="ExternalInput")
    xu = nc.dram_tensor("xu", [128, H, NW], BF16, kind="ExternalInput")
    wu = nc.dram_tensor("wu", [6, 128, 128], BF16, kind="ExternalInput")
    w4 = nc.dram_tensor("w4", [3, 128, 128], BF16, kind="ExternalInput")
    bias = nc.dram_tensor("bias", [128, 1], F32, kind="ExternalInput")
    # out[p, f, i, n, w] = conv[n, f, 2*i + p, w]  (bf16; host upcasts)
    out = nc.dram_tensor("out", [2, F, HO // 2, NB, WO], BF16,
                         kind="ExternalOutput")

    with TileContext(nc) as tc:
        with (
            tc.tile_pool(name="wp", bufs=1) as wp,
            tc.tile_pool(name="bp", bufs=1) as bp,
            tc.tile_pool(name="gp", bufs=gbufs) as gp,
            tc.tile_pool(name="up", bufs=ubufs) as up,
            tc.tile_pool(name="pp", bufs=psbufs, space="PSUM") as pp,
            tc.tile_pool(name="op", bufs=stbufs) as op,
        ):
            wut, w4t = [], []
            for k in range(6):
                t = wp.tile([128, 128], BF16, tag=f"wu{k}")
                nc.scalar.dma_start(out=t[:], in_=wu[k, :, :])
                wut.append(t)
            for k in range(3):
                t = wp.tile([128, 128], BF16, tag=f"w4_{k}")
                nc.scalar.dma_start(out=t[:], in_=w4[k, :, :])
                w4t.append(t)
            bt = bp.tile([128, 1], F32)
            nc.scalar.dma_start(out=bt[:], in_=bias[:])

            uslab, gslab = {}, {}

            def load_uslab(k):
                t = up.tile([128, RU * NW], BF16, tag="xu")
                nc.sync.dma_start(
                    out=t[:],
                    in_=xu[:, RU * k:RU * k + RU, :].rearrange(
                        "p r w -> p (r w)"))
                uslab[k] = t

            def load_gslab(k):
                t = gp.tile([128, RB * NW], BF16, tag="xg")
                nc.sync.dma_start(
                    out=t[:],
                    in_=xg4[:, RB * k:RB * k + RB, :].rearrange(
                        "p b w -> p (b w)"))
                gslab[k] = t

            def urhs(r):
                return uslab[r // RU][:, (r % RU) * NW:(r % RU) * NW + NW]

            def grhs(b, p0, p1):
                base = (b % RB) * NW
                return gslab[b // RB][p0:p1, base:base + NW]

            def load_uslab0_split():
                t = up.tile([128, RU * NW], BF16, tag="xu")
                half = (RU // 2) * NW
                nc.sync.dma_start(
                    out=t[:, 0:half],
                    in_=xu[:, 0:RU // 2, :].rearrange("p r w -> p (r w)"))
                nc.sync.dma_start(
                    out=t[:, half:RU * NW],
                    in_=xu[:, RU // 2:RU, :].rearrange("p r w -> p (r w)"))
                uslab[0] = t

            def emit_pass():
                uslab.clear()
                gslab.clear()
                load_uslab0_split()
                load_gslab(0)
                for k in range(1, 4):
                    load_uslab(k)
                for k in range(1, 3):
                    load_gslab(k)
                stage = None
                for h in range(0, HO - 1, 2):
                    b, phase = h // 4, h % 4
                    if h % RU == 0 and h // RU + 4 < NSU:
                        load_uslab(h // RU + 4)
                    if phase == 0 and b % RB == 0 and b // RB + 3 < NSB:
                        load_gslab(b // RB + 3)
                    if h % GH == 0:
                        stage = op.tile([128, (GH // 2) * NW], BF16,
                                        tag="stage")
                    ci = ((h % GH) // 2) * NW

                    ps = pp.tile([128, NW], F32, tag="ps")
                    # kw 0..3: rows h..h+5 via pre-shifted xu slabs
                    for k in range(6):
                        nc.tensor.matmul(
                            out=ps[:, 0:NW], lhsT=wut[k][:],
                            rhs=urhs(h + k),
                            start=(k == 0), stop=False)
                    # kw=4 from the block grid, rhs shifted by 4
                    if phase == 0:
                        nc.tensor.matmul(
                            out=ps[:, 0:NW], lhsT=w4t[0][:],
                            rhs=grhs(b, 0, 128),
                            start=False, stop=False)
                        nc.tensor.matmul(
                            out=ps[:, 0:NW], lhsT=w4t[2][0:64, :],
                            rhs=grhs(b + 1, 0, 64),
                            start=False, stop=True,
                            tile_position=(0, 0))
                    else:
                        nc.tensor.matmul(
                            out=ps[:, 0:NW], lhsT=w4t[1][:],
                            rhs=grhs(b + 1, 0, 128),
                            start=False, stop=False)
                        nc.tensor.matmul(
                            out=ps[:, 0:NW], lhsT=w4t[2][64:128, :],
                            rhs=grhs(b, 64, 128),
                            start=False, stop=True,
                            tile_position=(64, 0))

                    nc.scalar.activation(
                        out=stage[:, ci:ci + NW], in_=ps[:, 0:NW],
                        func=mybir.ActivationFunctionType.Identity,
                        bias=bt[:], scale=1.0)

                    if h % GH == GH - 2:
                        i0 = (h - (GH - 2)) // 2
                        eng = nc.scalar if (h // GH) % 2 == 0 else nc.sync
                        eng.dma_start(
                            out=out[:, :, i0:i0 + GH // 2, :, :].rearrange(
                                "p f i n w -> (p f) (i n w)"),
                            in_=stage[:],
                        )

            if loop_n:
                with tc.For_i(0, loop_n):
                    emit_pass()
            else:
                for _ in range(reps):
                    emit_pass()

    nc.finalize()
    return nc


def _prep_weights(weight):
    """wu[k][32s+c, 64p+f] = weight[f, c, k-p, s]   (s=kw 0..3, k=0..5)
    w4[0/1/2]: kw=4 phase-0 main / phase-2 main / strips (see v3).
    All out-of-range kh -> 0.
    """
    wu = np.zeros((6, 128, 128), np.float32)
    w4 = np.zeros((3, 128, 128), np.float32)
    for p in range(2):
        for k in range(6):
            kh = k - p
            if 0 <= kh < K:
                for s in range(4):
                    wu[k, 32 * s:32 * s + 32, 64 * p:64 * p + 64] = \
                        weight[:, :, kh, s].T
        for j in range(4):
            kh = j - p
            if 0 <= kh < K:
                w4[0, 32 * j:32 * j + 32, 64 * p:64 * p + 64] = \
                    weight[:, :, kh, 4].T
            kh = j + 2 - p
            if 0 <= kh < K:
                w4[1, 32 * j:32 * j + 32, 64 * p:64 * p + 64] = \
                    weight[:, :, kh, 4].T
            kh = (j + 4 - p) if j < 2 else (j - 2 - p)
            if 0 <= kh < K:
                w4[2, 32 * j:32 * j + 32, 64 * p:64 * p + 64] = \
                    weight[:, :, kh, 4].T
    return wu.astype(BF16NP), w4.astype(BF16NP)


def _prep_core(xs, weight, bias, wbanks=None):
    """xs: (2,32,224,224) -> per-core input map."""
    # xu[32s+c, r, 220n+w] = x[n, c, r, w+s]  (s=0..3, w=0..219)
    xu = np.empty((4, C, H, NB, WO), np.float32)
    for s in range(4):
        xu[s] = xs[:, :, :, s:s + WO].transpose(1, 2, 0, 3)
    xu = np.ascontiguousarray(xu.reshape(128, H, NW)).astype(BF16NP)

    # xg4[32j+c, b, 220n+w] = x[n, c, 4b+j, w+4]
    a4 = xs[:, :, :, 4:4 + WO].transpose(1, 2, 0, 3)  # [c, r, n, w]
    xg4 = np.ascontiguousarray(
        a4.reshape(C, NBLK, 4, NB, WO).transpose(2, 0, 1, 3, 4).reshape(
            128, NBLK, NW)).astype(BF16NP)

    if wbanks is None:
        wbanks = _prep_weights(weight)
    wu, w4 = wbanks
    b128 = np.concatenate([bias, bias]).reshape(128, 1).astype(np.float32)
    return {"xg4": xg4, "xu": xu, "wu": wu, "w4": w4, "bias": b128}


def kernel(x, weight, bias, _profile=False):
    x = np.asarray(x, dtype=np.float32)
    weight = np.asarray(weight, dtype=np.float32)
    bias = np.asarray(bias, dtype=np.float32)

    if "nc" not in _cache:
        _cache["nc"] = _build()
    nc = _cache["nc"]

    wbanks = _prep_weights(weight)
    in_maps = [
        _prep_core(x[NB * i:NB * i + NB], weight, bias, wbanks)
        for i in range(N_CORES)
    ]
    res = run_bass_kernel_spmd(
        nc, in_maps, core_ids=list(range(N_CORES)), trace=_profile)
    outs = []
    for r in res.results:
        ot = np.asarray(r["out"]).astype(np.float32).reshape(
            2, F, HO // 2, NB, WO)
        outs.append(
            np.ascontiguousarray(ot.transpose(3, 1, 2, 0, 4)).reshape(
                NB, F, HO, WO))
    out = np.concatenate(outs, axis=0)
    if _profile:
        _cache["last_results"] = res
    return out


def _postprocess_raw(raw):
    """Concatenated raw dram outs (8*2,64,110,2,220) -> (16,64,220,220)."""
    raw = np.asarray(raw).astype(np.float32).reshape(
        N_CORES, 2, F, HO // 2, NB, WO)
    return np.ascontiguousarray(raw.transpose(0, 4, 2, 3, 1, 5)).reshape(
        N_CORES * NB, F, HO, WO)


if __name__ == "__main__":
    rng = np.random.default_rng(0)
    x = rng.standard_normal((B, C, H, W), dtype=np.float32)
    w = rng.standard_normal((F, C, K, K), dtype=np.float32)
    bv = rng.standard_normal((F,), dtype=np.float32)
    o = kernel(x, w, bv)
    print("output shape:", o.shape, o.dtype)

